# revision 11
# baseline (speedup 1.0000x reference)
"""Trainium2 Bass kernel for a 2-layer ChebConv (K=5) GNN + global_add_pool + fc.

Strategy (8 NeuronCores, SPMD):
  - dst-shard the edges: core c owns all edges whose dst lands in its node
    shard. Each hop's scatter output is then complete per-core (no
    all-reduce); cores exchange an fp16 node-feature table via AllGather.
  - Gather messages with dma_gather (256B elements = two consecutive fp16
    rows of the node table; per-edge parity selects which half, handled by
    splitting each 128-edge chunk into two 64-row K-ranges on the PE).
  - Scatter-add via PE matmul: per chunk, out[feat, node_window] +=
    G.T @ S where S is a host-precomputed fp16 selection matrix with the
    edge weight (-1/deg[src]) folded in.
  - Chebyshev recurrence (Tx2 = 2*prop(Tx1) - Tx0) is fused: the PSUM is
    seeded with -Tx0/2 via an f32r matmul and the evacuation multiplies
    by 2.  Per-node W matmuls run as f32r with the fp32 state as rhs.
"""

import os
import sys
import numpy as np

for _p in ("/opt/trn_rl_repo",):
    if os.path.isdir(_p) and _p not in sys.path:
        sys.path.insert(0, _p)

# ---------------------------------------------------------------- config

SELU_L = 1.0507009873554805
SELU_A = 1.6732632423543772


class Cfg:
    def __init__(self, N=100_000, E=1_250_000, NG=64, F=64, K=5, OUT=10,
                 NCORES=8, call_chunks=32):
        self.N, self.E, self.NG, self.F, self.K, self.OUT = N, E, NG, F, K, OUT
        self.NCORES = NCORES
        self.SHARD = (N + NCORES - 1) // NCORES
        self.PSHARD = ((self.SHARD + 127) // 128) * 128
        self.TBL = NCORES * self.PSHARD          # padded table rows
        self.NBLK = (self.PSHARD + 511) // 512   # psum blocks per shard
        self.WIN = 64                            # scatter window (nodes)
        self.NBUCKET = 2 if self.TBL > 65536 else 1
        self.CALL_CHUNKS = call_chunks           # chunks per dma_gather call


# ---------------------------------------------------------------- host plan


def build_plan(cfg, edge_index):
    """Global (core-independent) chunk structure + per-core S/idx arrays."""
    N, NC = cfg.N, cfg.NCORES
    src = np.asarray(edge_index[0], dtype=np.int64)
    dst = np.asarray(edge_index[1], dtype=np.int64)
    deg = np.bincount(src, minlength=N).astype(np.float64)
    ew = (-1.0 / deg[src]).astype(np.float32)

    # table row of each src node (shards padded to PSHARD rows)
    r_src = (src // cfg.SHARD) * cfg.PSHARD + (src % cfg.SHARD)
    core = dst // cfg.SHARD
    dl = dst % cfg.SHARD                      # dst local node id
    blk = dl // 512
    par = (r_src & 1).astype(np.int64)        # parity within row pair
    pair = r_src >> 1
    bkt = (pair >= 32768).astype(np.int64) if cfg.NBUCKET == 2 else np.zeros_like(par)
    m = np.where(bkt == 1, pair - 32768, pair).astype(np.int64)  # bucket-local

    # group edges by (core, block, bucket), dst-sorted inside; chunks mix
    # parities (per-slot parity masks select the gathered pair half on-chip)
    key = ((core * cfg.NBLK + blk) * cfg.NBUCKET + bkt) * cfg.SHARD + dl
    order = np.argsort(key, kind="stable")
    g_dl, g_m, g_ew, g_par = dl[order], m[order], ew[order], par[order]
    gk = key[order] // cfg.SHARD  # group id per sorted edge
    ngroups = NC * cfg.NBLK * cfg.NBUCKET
    starts = np.searchsorted(gk, np.arange(ngroups + 1))

    def grp(c, b, bk):
        gid = (c * cfg.NBLK + b) * cfg.NBUCKET + bk
        s, e = starts[gid], starts[gid + 1]
        return g_dl[s:e], g_m[s:e], g_ew[s:e], g_par[s:e]

    blocks_meta = []   # [b][bkt] -> dict(calls=[(cstart,nch)], wins=[w...])
    CT = 0
    # per-core chunk piece lists: (chunk_id, dsts, ms, ews, pars) appended
    core_chunks = [[] for _ in range(NC)]

    for b in range(cfg.NBLK):
        bmeta = []
        for bk in range(cfg.NBUCKET):
            data = [grp(c, b, bk) for c in range(NC)]
            ptr = [0 for _ in range(NC)]
            wins = []   # w0 per chunk
            run_start = CT
            while True:
                wmin = None
                for c in range(NC):
                    d = data[c][0]
                    if ptr[c] < len(d):
                        v = d[ptr[c]]
                        if wmin is None or v < wmin:
                            wmin = v
                if wmin is None:
                    break
                wb_psum = min(512, cfg.PSHARD - b * 512)
                w0 = min(int(wmin) - b * 512,      # window base within block
                         max(0, wb_psum - cfg.WIN))
                limit = b * 512 + w0 + cfg.WIN
                cid = CT
                for c in range(NC):
                    d, mm, ee, pp = data[c]
                    lo = ptr[c]
                    hi = np.searchsorted(d, limit, side="left")
                    take = min(128, hi - lo)
                    if take > 0:
                        core_chunks[c].append(
                            (cid, d[lo:lo + take] - b * 512 - w0,
                             mm[lo:lo + take], ee[lo:lo + take],
                             pp[lo:lo + take]))
                        ptr[c] = lo + take
                wins.append(w0)
                CT += 1
            nch_run = CT - run_start
            calls = []
            off = 0
            while off < nch_run:
                n = min(cfg.CALL_CHUNKS, nch_run - off)
                calls.append((run_start + off, n))
                off += n
            bmeta.append({"calls": calls, "wins": wins, "cstart": run_start,
                          "nch": nch_run})
        blocks_meta.append(bmeta)

    # materialize per-core arrays
    S_list, idx_list = [], []
    for c in range(NC):
        S = np.zeros((CT, 2, 128, cfg.WIN), dtype=np.float16)
        idx = np.zeros((CT, 128), dtype=np.int16)
        for cid, dls, ms, ees, pps in core_chunks[c]:
            n = len(dls)
            rows = np.arange(n)
            S[cid, pps, rows, dls] = ees.astype(np.float16)
            idx[cid, rows] = ms.astype(np.int16)
        # wrap idx per call: position i -> [i%16, i//16]
        idx_w = np.zeros((16, CT * 8), dtype=np.int16)
        for bmeta in blocks_meta:
            for bm in bmeta:
                for (cs, n) in bm["calls"]:
                    flat = idx[cs:cs + n].reshape(-1)           # [n*128]
                    idx_w[:, cs * 8:(cs + n) * 8] = flat.reshape(-1, 16).T
        # S flat: [128, CT*2*WIN] (even-parity S then odd-parity S per chunk)
        S_flat = np.ascontiguousarray(
            S.transpose(2, 0, 1, 3).reshape(128, CT * 2 * cfg.WIN))
        S_list.append(S_flat)
        idx_list.append(np.ascontiguousarray(np.tile(idx_w, (8, 1))))  # [128, CT*8]

    return {"CT": CT, "blocks": blocks_meta, "S": S_list, "idx": idx_list}


def build_host_inputs(cfg, plan, x, batch, W1, b1, W2, b2, Wfc, bfc):
    """Per-core in_map dicts."""
    N, F, NG = cfg.N, cfg.F, cfg.NG
    x = np.asarray(x, np.float32)
    batch = np.asarray(batch, np.int64)
    table0 = np.zeros((cfg.TBL, F), np.float16)
    W_sb = np.zeros((128, 2 * cfg.K * F), np.float32)
    for l, W in enumerate((W1, W2)):
        for k in range(cfg.K):
            blkc = (l * cfg.K + k) * F
            r0 = (k % 2) * 64          # W_k contracts state rows of Tx_k
            W_sb[r0:r0 + 64, blkc:blkc + F] = W[k]
    b12 = np.stack([np.asarray(b1, np.float32), np.asarray(b2, np.float32)], axis=1)
    ident = np.zeros((128, 64), np.float32)
    ident[np.arange(128), np.arange(128) % 64] = 1.0
    # neghalf[:, hc:hc+64] = -0.5*I placed on rows hc:hc+64, zeros elsewhere
    neghalf = np.zeros((128, 128), np.float32)
    neghalf[np.arange(64), np.arange(64)] = -0.5
    neghalf[np.arange(64, 128), np.arange(64, 128)] = -0.5
    ngrp = cfg.PSHARD // 128

    in_maps = []
    for c in range(cfg.NCORES):
        lo, hi = c * cfg.SHARD, min((c + 1) * cfg.SHARD, N)
        ns = hi - lo
        table0[c * cfg.PSHARD:c * cfg.PSHARD + ns] = x[lo:hi].astype(np.float16)
    for c in range(cfg.NCORES):
        lo, hi = c * cfg.SHARD, min((c + 1) * cfg.SHARD, N)
        ns = hi - lo
        x_fm = np.zeros((64, cfg.PSHARD), np.float32)
        x_fm[:, :ns] = x[lo:hi].T
        bt = np.zeros((128, ngrp * NG), np.float16)
        l_ = np.arange(ns)
        bt[l_ % 128, (l_ // 128) * NG + batch[lo:hi]] = 1.0
        in_maps.append({
            "x_fm": x_fm,
            "table0": table0,
            "s_all": plan["S"][c],
            "idx_all": plan["idx"][c],
            "bt_in": bt,
            "w_sb_in": W_sb,
            "b12_in": b12,
            "wfc_in": np.asarray(Wfc, np.float32),
            "bfc_in": np.asarray(bfc, np.float32).reshape(cfg.OUT, 1),
            "ident_in": ident,
            "neghalf_in": neghalf,
        })
    return in_maps


# ---------------------------------------------------------------- device


def build_kernel(cfg, plan, nprop=None, debug=False):
    import concourse.bass as bass
    import concourse.bacc as bacc
    import concourse.mybir as mybir
    import concourse.tile as tile

    dt = mybir.dt
    F, K, NG, OUT = cfg.F, cfg.K, cfg.NG, cfg.OUT
    PSH, TBL, CT, WIN = cfg.PSHARD, cfg.TBL, plan["CT"], cfg.WIN
    NBLK = cfg.NBLK
    ngrp = PSH // 128

    nc = bacc.Bacc("TRN2", debug=False, target_bir_lowering=False,
                   num_devices=cfg.NCORES,
                   dynamic_dma_scratch_size=40960)

    # I/O
    x_fm_t = nc.dram_tensor("x_fm", [64, PSH], dt.float32, kind="ExternalInput")
    table0_t = nc.dram_tensor("table0", [TBL, F], dt.float16, kind="ExternalInput")
    s_all_t = nc.dram_tensor("s_all", [128, CT * 2 * WIN], dt.float16, kind="ExternalInput")
    idx_all_t = nc.dram_tensor("idx_all", [128, CT * 8], dt.int16, kind="ExternalInput")
    bt_t = nc.dram_tensor("bt_in", [128, ngrp * NG], dt.float16, kind="ExternalInput")
    w_sb_t = nc.dram_tensor("w_sb_in", [128, 2 * K * F], dt.float32, kind="ExternalInput")
    b12_t = nc.dram_tensor("b12_in", [64, 2], dt.float32, kind="ExternalInput")
    wfc_t = nc.dram_tensor("wfc_in", [64, OUT], dt.float32, kind="ExternalInput")
    bfc_t = nc.dram_tensor("bfc_in", [OUT, 1], dt.float32, kind="ExternalInput")
    ident_t = nc.dram_tensor("ident_in", [128, 64], dt.float32, kind="ExternalInput")
    neghalf_t = nc.dram_tensor("neghalf_in", [128, 128], dt.float32, kind="ExternalInput")
    out_t = nc.dram_tensor("out_t", [OUT, NG], dt.float32, kind="ExternalOutput")
    if debug:
        dbg0_t = nc.dram_tensor("dbg0", [64, PSH], dt.float32, kind="ExternalOutput")
        dbg1_t = nc.dram_tensor("dbg1", [64, PSH], dt.float32, kind="ExternalOutput")
        dbgo_t = nc.dram_tensor("dbgo", [64, PSH], dt.float32, kind="ExternalOutput")

    f32r = dt.float32r
    rg = [list(range(cfg.NCORES))]
    skip_gather = bool(int(os.environ.get("KSKIP_GATHER", "0")))
    skip_ag = bool(int(os.environ.get("KSKIP_AG", "0")))
    skip_trans = bool(int(os.environ.get("KSKIP_TRANS", "0")))

    with tile.TileContext(nc) as tc:
        with (
            tc.tile_pool(name="const", bufs=1) as cpool,
            tc.tile_pool(name="state", bufs=1) as spool,
            tc.tile_pool(name="gather", bufs=2) as gpool,
            tc.tile_pool(name="smat", bufs=2) as smpool,
            tc.tile_pool(name="idx", bufs=2) as ipool,
            tc.tile_pool(name="psum_y", bufs=2, space="PSUM") as pyp,
            tc.tile_pool(name="psum_w", bufs=2, space="PSUM") as pwp,
            tc.tile_pool(name="psum_t", bufs=2, space="PSUM") as ptp,
            tc.tile_pool(name="dram", bufs=1, space="DRAM") as dpool,
        ):
            # ---- constants to SBUF
            w_sb = cpool.tile([128, 2 * K * F], dt.float32)
            b12_sb = cpool.tile([64, 2], dt.float32)
            wfc_sb = cpool.tile([64, OUT], dt.float32)
            bfc_sb = cpool.tile([OUT, 1], dt.float32)
            ident_sb = cpool.tile([128, 64], dt.float32)
            neghalf_sb = cpool.tile([128, 128], dt.float32)
            bt_sb = cpool.tile([128, ngrp * NG], dt.float16)
            nc.sync.dma_start(out=w_sb[:], in_=w_sb_t[:])
            nc.sync.dma_start(out=b12_sb[:], in_=b12_t[:])
            nc.sync.dma_start(out=wfc_sb[:], in_=wfc_t[:])
            nc.sync.dma_start(out=bfc_sb[:], in_=bfc_t[:])
            nc.sync.dma_start(out=ident_sb[:], in_=ident_t[:])
            nc.sync.dma_start(out=neghalf_sb[:], in_=neghalf_t[:])
            nc.sync.dma_start(out=bt_sb[:], in_=bt_t[:])

            # ---- state
            stA = spool.tile([128, PSH], dt.float32)   # halves: Tx(k even) / Tx(k odd)
            out_sb = spool.tile([64, PSH], dt.float32)
            nm_sb = spool.tile([128, ngrp * F], dt.float16)
            p_sb = spool.tile([64, 512], dt.float32)   # selu pos part scratch
            g_sb = spool.tile([64, NG], dt.float32)
            gfull_sb = spool.tile([64, NG], dt.float32)
            o_sb = spool.tile([OUT, NG], dt.float32)

            nc.sync.dma_start(out=stA[0:64, :], in_=x_fm_t[:])

            # ---- DRAM
            tbuf0 = dpool.tile([TBL, F], dt.float16, tag="tbuf0")
            tbuf1 = dpool.tile([TBL, F], dt.float16, tag="tbuf1")
            tbuf = [tbuf0, tbuf1]
            stage_in = dpool.tile([PSH, F], dt.float16)
            gt_in = dpool.tile([64, NG], dt.float32)
            gt_out = dpool.tile([64, NG], dt.float32)

            def gather_src(h, bk):
                """in_ap for dma_gather: bucket bk of the table for prop h."""
                base = 0 if bk == 0 else 65536
                rows = min(TBL, 65536) if bk == 0 else TBL - 65536
                t = table0_t if h == 0 else tbuf[h % 2]
                ap = t[base:base + rows, :] if h == 0 else t[base:base + rows, :]
                return ap.rearrange("(r two) f -> r (two f)", two=2)

            NPROP = 2 * (K - 1) if nprop is None else nprop
            for h in range(NPROP):
                l, k = h // (K - 1), h % (K - 1) + 1
                hc = (k % 2) * 64          # partition base of Tx_k
                hp = 64 - hc               # partition base of Tx_{k-2 / k-1}

                for b in range(NBLK):
                    w_b = min(512, PSH - b * 512)
                    bc = slice(b * 512, b * 512 + w_b)
                    psum_y = pyp.tile([128, 512], dt.float32)
                    if k == 1:
                        nc.vector.memset(psum_y[hc:hc + 64, :w_b], 0.0)
                    else:
                        # psum := -Tx_{k-2}/2  (Tx_{k-2} shares half hc with Tx_k;
                        # K=128 with zero rows keeps tile_position uniform)
                        nc.tensor.matmul(
                            psum_y[hc:hc + 64, :w_b],
                            neghalf_sb[:, hc:hc + 64],
                            stA[:, bc],
                            start=True, stop=False, skip_group_check=True)
                    for bk in range(cfg.NBUCKET if not skip_gather else 0):
                        bm = plan["blocks"][b][bk]
                        src_ap = gather_src(h, bk)
                        for (cs, nch) in bm["calls"]:
                            it = ipool.tile([128, cfg.CALL_CHUNKS * 8], dt.int16)
                            st = smpool.tile([128, cfg.CALL_CHUNKS * 2 * WIN], dt.float16)
                            gt = gpool.tile([128, cfg.CALL_CHUNKS * 128], dt.float16)
                            nc.sync.dma_start(out=it[:, :nch * 8],
                                              in_=idx_all_t[:, cs * 8:(cs + nch) * 8])
                            nc.sync.dma_start(
                                out=st[:, :nch * 2 * WIN],
                                in_=s_all_t[:, cs * 2 * WIN:(cs + nch) * 2 * WIN])
                            L = nch * 128
                            nc.gpsimd.dma_gather(
                                gt[:, :L].rearrange("p (c f) -> p c f", f=128),
                                src_ap, it[:, :nch * 8], L, L, 128,
                                single_packet=False)
                            for j in range(nch):
                                w = bm["wins"][cs - bm["cstart"] + j]
                                last = (bk == cfg.NBUCKET - 1
                                        and cs + j == bm["cstart"] + bm["nch"] - 1)
                                c0 = j * 128
                                sb = j * 2 * WIN
                                nc.tensor.matmul(
                                    psum_y[hc:hc + 64, w:w + WIN],
                                    gt[:, c0:c0 + 64],
                                    st[:, sb:sb + WIN],
                                    start=False, stop=False, skip_group_check=True)
                                nc.tensor.matmul(
                                    psum_y[hc:hc + 64, w:w + WIN],
                                    gt[:, c0 + 64:c0 + 128],
                                    st[:, sb + WIN:sb + 2 * WIN],
                                    start=False, stop=last, skip_group_check=True)
                    # evacuate: Tx_k
                    if k == 1:
                        nc.vector.tensor_copy(stA[hc:hc + 64, bc], psum_y[hc:hc + 64, :w_b])
                    else:
                        nc.vector.tensor_scalar(stA[hc:hc + 64, bc], psum_y[hc:hc + 64, :w_b],
                                                2.0, None, mybir.AluOpType.mult)
                    # out += Tx_k @ W_k  (transposed: psum_w = W_k.T @ Tx_k)
                    psum_w = pwp.tile([64, 512], dt.float32)
                    wc = (l * K + k) * F
                    nc.tensor.matmul(psum_w[:, :w_b],
                                     w_sb[:, wc:wc + F],
                                     stA[:, bc],
                                     start=True, stop=(k != 1), skip_group_check=True)
                    if k == 1:  # also Tx0 @ W0
                        nc.tensor.matmul(psum_w[:, :w_b],
                                         w_sb[:, (l * K) * F:(l * K) * F + F],
                                         stA[:, bc],
                                         start=False, stop=True, skip_group_check=True)
                        nc.vector.tensor_copy(out_sb[:, bc], psum_w[:, :w_b])
                    else:
                        nc.vector.tensor_tensor(out_sb[:, bc], out_sb[:, bc],
                                                psum_w[:, :w_b], mybir.AluOpType.add)

                hsrc = hc  # partition base of features to tableize
                if k == K - 1:
                    # ---- h = selu(out + b_l) -> stA[0:64]
                    lam, alpha = SELU_L, SELU_A
                    for b in range(NBLK):
                        w_b = min(512, PSH - b * 512)
                        bc = slice(b * 512, b * 512 + w_b)
                        nc.vector.tensor_scalar(out_sb[:, bc], out_sb[:, bc],
                                                b12_sb[:, l:l + 1], None,
                                                mybir.AluOpType.add)
                        nc.scalar.activation(p_sb[:, :w_b], out_sb[:, bc],
                                             mybir.ActivationFunctionType.Relu,
                                             scale=lam)
                        nc.vector.tensor_scalar(out_sb[:, bc], out_sb[:, bc],
                                                0.0, None, mybir.AluOpType.min)
                        nc.scalar.activation(out_sb[:, bc], out_sb[:, bc],
                                             mybir.ActivationFunctionType.Exp)
                        nc.vector.tensor_scalar(out_sb[:, bc], out_sb[:, bc],
                                                lam * alpha, -lam * alpha,
                                                mybir.AluOpType.mult,
                                                mybir.AluOpType.add)
                        nc.vector.tensor_tensor(stA[0:64, bc], out_sb[:, bc],
                                                p_sb[:, :w_b], mybir.AluOpType.add)
                    hsrc = 0

                # ---- build node-major fp16 table from stA[hsrc:hsrc+64]
                if not skip_trans:
                    for g in range(ngrp):
                        pt = ptp.tile([128, 64], dt.float32)
                        nc.tensor.matmul(pt[:],
                                         stA[hsrc:hsrc + 64, g * 128:(g + 1) * 128],
                                         ident_sb[hsrc:hsrc + 64, :],
                                         is_transpose=True, skip_group_check=True)
                        nc.vector.tensor_copy(nm_sb[:, g * F:(g + 1) * F], pt[:])
                if not (l == 1 and k == K - 1) and not skip_ag and not skip_trans:
                    # ship shard and all-gather into the other table buffer
                    nc.sync.dma_start(
                        out=stage_in[:].rearrange("(g p) f -> p g f", p=128),
                        in_=nm_sb[:].rearrange("p (g f) -> p g f", f=F))
                    nc.gpsimd.collective_compute(
                        "AllGather", mybir.AluOpType.bypass,
                        replica_groups=rg,
                        ins=[stage_in.opt()],
                        outs=[tbuf[(h + 1) % 2].opt()])

            if debug:
                nc.sync.dma_start(out=dbg0_t[:], in_=stA[0:64, :])
                nc.sync.dma_start(out=dbg1_t[:], in_=stA[64:128, :])
                nc.sync.dma_start(out=dbgo_t[:], in_=out_sb[:])
            # ---- pooling: gT = sum_n h2[n] per graph  (psum[64f, NG])
            if not skip_trans:
                psum_g = pwp.tile([64, 512], dt.float32, tag="psum_w")
                for g in range(ngrp):
                    nc.tensor.matmul(psum_g[:, :NG],
                                     nm_sb[:, g * F:(g + 1) * F],
                                     bt_sb[:, g * NG:(g + 1) * NG],
                                     start=(g == 0), stop=(g == ngrp - 1),
                                     skip_group_check=True)
                nc.vector.tensor_copy(g_sb[:], psum_g[:, :NG])
            else:
                nc.vector.memset(g_sb[:], 0.0)
            nc.sync.dma_start(out=gt_in[:], in_=g_sb[:])
            nc.gpsimd.collective_compute(
                "AllReduce", mybir.AluOpType.add, replica_groups=rg,
                ins=[gt_in.opt()], outs=[gt_out.opt()])
            nc.sync.dma_start(out=gfull_sb[:], in_=gt_out[:])
            psum_o = ptp.tile([128, 64], dt.float32, tag="pt")
            nc.tensor.matmul(psum_o[0:OUT, 0:NG],
                             wfc_sb[:],
                             gfull_sb[:],
                             start=True, stop=True, skip_group_check=True)
            nc.vector.tensor_scalar(o_sb[:], psum_o[0:OUT, 0:NG],
                                    bfc_sb[:, 0:1], None, mybir.AluOpType.add)
            nc.sync.dma_start(out=out_t[:], in_=o_sb[:])

    nc.compile()
    return nc


# ---------------------------------------------------------------- entry


def run(cfg, inputs, trace=False):
    from concourse.bass_utils import run_bass_kernel_spmd
    edge_index = np.asarray(inputs["edge_index"])
    plan = build_plan(cfg, edge_index)
    nprop = int(os.environ.get("KNPROP", "0")) or None
    nc = build_kernel(cfg, plan, nprop=nprop)
    in_maps = build_host_inputs(
        cfg, plan, inputs["x"], inputs["batch"],
        inputs["W1"], inputs["b1"], inputs["W2"], inputs["b2"],
        inputs["Wfc"], inputs["bfc"])
    core_ids = list(range(cfg.NCORES))
    res = run_bass_kernel_spmd(nc, in_maps, core_ids, trace=trace)
    out = np.asarray(res.results[0]["out_t"]).T.copy()  # [NG, OUT]
    return out, res


def kernel(**inputs):
    cfg = Cfg()
    out, _ = run(cfg, inputs, trace=False)
    return out.astype(np.float32)



# revision 12
# speedup vs baseline: 1.1066x; 1.1066x over previous
"""Trainium2 Bass kernel for a 2-layer ChebConv (K=5) GNN + global_add_pool + fc.

Strategy (8 NeuronCores, SPMD):
  - dst-shard the edges: core c owns all edges whose dst lands in its node
    shard. Each hop's scatter output is then complete per-core (no
    all-reduce); cores exchange an fp16 node-feature table via AllGather.
  - Gather messages with dma_gather (256B elements = two consecutive fp16
    rows of the node table; per-edge parity selects which half, handled by
    splitting each 128-edge chunk into two 64-row K-ranges on the PE).
  - Scatter-add via PE matmul: per chunk, out[feat, node_window] +=
    G.T @ S where S is a host-precomputed fp16 selection matrix with the
    edge weight (-1/deg[src]) folded in.
  - Chebyshev recurrence (Tx2 = 2*prop(Tx1) - Tx0) is fused: the PSUM is
    seeded with -Tx0/2 via an f32r matmul and the evacuation multiplies
    by 2.  Per-node W matmuls run as f32r with the fp32 state as rhs.
"""

import os
import sys
import numpy as np

for _p in ("/opt/trn_rl_repo",):
    if os.path.isdir(_p) and _p not in sys.path:
        sys.path.insert(0, _p)

# ---------------------------------------------------------------- config

SELU_L = 1.0507009873554805
SELU_A = 1.6732632423543772


class Cfg:
    def __init__(self, N=100_000, E=1_250_000, NG=64, F=64, K=5, OUT=10,
                 NCORES=8, call_chunks=16):
        self.N, self.E, self.NG, self.F, self.K, self.OUT = N, E, NG, F, K, OUT
        self.NCORES = NCORES
        self.SHARD = (N + NCORES - 1) // NCORES
        self.PSHARD = ((self.SHARD + 127) // 128) * 128
        self.TBL = NCORES * self.PSHARD          # padded table rows
        self.NBLK = (self.PSHARD + 511) // 512   # psum blocks per shard
        self.WIN = 64                            # scatter window (nodes)
        self.NBUCKET = 2 if self.TBL > 65536 else 1
        self.CALL_CHUNKS = call_chunks           # chunks per dma_gather call


# ---------------------------------------------------------------- host plan


def build_plan(cfg, edge_index):
    """Global (core-independent) chunk structure + per-core S/idx arrays."""
    N, NC = cfg.N, cfg.NCORES
    src = np.asarray(edge_index[0], dtype=np.int64)
    dst = np.asarray(edge_index[1], dtype=np.int64)
    deg = np.bincount(src, minlength=N).astype(np.float64)
    ew = (-1.0 / deg[src]).astype(np.float32)

    # table row of each src node (shards padded to PSHARD rows)
    r_src = (src // cfg.SHARD) * cfg.PSHARD + (src % cfg.SHARD)
    core = dst // cfg.SHARD
    dl = dst % cfg.SHARD                      # dst local node id
    blk = dl // 512
    par = (r_src & 1).astype(np.int64)        # parity within row pair
    pair = r_src >> 1
    bkt = (pair >= 32768).astype(np.int64) if cfg.NBUCKET == 2 else np.zeros_like(par)
    m = np.where(bkt == 1, pair - 32768, pair).astype(np.int64)  # bucket-local

    # group edges by (core, block, bucket), dst-sorted inside; chunks mix
    # parities (per-slot parity masks select the gathered pair half on-chip)
    key = ((core * cfg.NBLK + blk) * cfg.NBUCKET + bkt) * cfg.SHARD + dl
    order = np.argsort(key, kind="stable")
    g_dl, g_m, g_ew, g_par = dl[order], m[order], ew[order], par[order]
    gk = key[order] // cfg.SHARD  # group id per sorted edge
    ngroups = NC * cfg.NBLK * cfg.NBUCKET
    starts = np.searchsorted(gk, np.arange(ngroups + 1))

    def grp(c, b, bk):
        gid = (c * cfg.NBLK + b) * cfg.NBUCKET + bk
        s, e = starts[gid], starts[gid + 1]
        return g_dl[s:e], g_m[s:e], g_ew[s:e], g_par[s:e]

    blocks_meta = []   # [b][bkt] -> dict(calls=[(cstart,nch)], wins=[w...])
    CT = 0
    # per-core chunk piece lists: (chunk_id, dsts, ms, ews, pars) appended
    core_chunks = [[] for _ in range(NC)]

    for b in range(cfg.NBLK):
        bmeta = []
        for bk in range(cfg.NBUCKET):
            data = [grp(c, b, bk) for c in range(NC)]
            ptr = [0 for _ in range(NC)]
            wins = []   # w0 per chunk
            run_start = CT
            while True:
                wmin = None
                for c in range(NC):
                    d = data[c][0]
                    if ptr[c] < len(d):
                        v = d[ptr[c]]
                        if wmin is None or v < wmin:
                            wmin = v
                if wmin is None:
                    break
                wb_psum = min(512, cfg.PSHARD - b * 512)
                w0 = min(int(wmin) - b * 512,      # window base within block
                         max(0, wb_psum - cfg.WIN))
                limit = b * 512 + w0 + cfg.WIN
                cid = CT
                for c in range(NC):
                    d, mm, ee, pp = data[c]
                    lo = ptr[c]
                    hi = np.searchsorted(d, limit, side="left")
                    take = min(128, hi - lo)
                    if take > 0:
                        core_chunks[c].append(
                            (cid, d[lo:lo + take] - b * 512 - w0,
                             mm[lo:lo + take], ee[lo:lo + take],
                             pp[lo:lo + take]))
                        ptr[c] = lo + take
                wins.append(w0)
                CT += 1
            nch_run = CT - run_start
            calls = []
            off = 0
            while off < nch_run:
                n = min(cfg.CALL_CHUNKS, nch_run - off)
                calls.append((run_start + off, n))
                off += n
            bmeta.append({"calls": calls, "wins": wins, "cstart": run_start,
                          "nch": nch_run})
        blocks_meta.append(bmeta)

    # materialize per-core arrays
    S_list, idx_list = [], []
    for c in range(NC):
        S = np.zeros((CT, 2, 128, cfg.WIN), dtype=np.float16)
        idx = np.zeros((CT, 128), dtype=np.int16)
        for cid, dls, ms, ees, pps in core_chunks[c]:
            n = len(dls)
            rows = np.arange(n)
            S[cid, pps, rows, dls] = ees.astype(np.float16)
            idx[cid, rows] = ms.astype(np.int16)
        # wrap idx per call: position i -> [i%16, i//16]
        idx_w = np.zeros((16, CT * 8), dtype=np.int16)
        for bmeta in blocks_meta:
            for bm in bmeta:
                for (cs, n) in bm["calls"]:
                    flat = idx[cs:cs + n].reshape(-1)           # [n*128]
                    idx_w[:, cs * 8:(cs + n) * 8] = flat.reshape(-1, 16).T
        # S flat: [128, CT*2*WIN] (even-parity S then odd-parity S per chunk)
        S_flat = np.ascontiguousarray(
            S.transpose(2, 0, 1, 3).reshape(128, CT * 2 * cfg.WIN))
        S_list.append(S_flat)
        idx_list.append(np.ascontiguousarray(np.tile(idx_w, (8, 1))))  # [128, CT*8]

    return {"CT": CT, "blocks": blocks_meta, "S": S_list, "idx": idx_list}


def build_host_inputs(cfg, plan, x, batch, W1, b1, W2, b2, Wfc, bfc):
    """Per-core in_map dicts."""
    N, F, NG = cfg.N, cfg.F, cfg.NG
    x = np.asarray(x, np.float32)
    batch = np.asarray(batch, np.int64)
    table0 = np.zeros((cfg.TBL, F), np.float16)
    W_sb = np.zeros((128, 2 * cfg.K * F), np.float32)
    for l, W in enumerate((W1, W2)):
        for k in range(cfg.K):
            blkc = (l * cfg.K + k) * F
            r0 = (k % 2) * 64          # W_k contracts state rows of Tx_k
            W_sb[r0:r0 + 64, blkc:blkc + F] = W[k]
    b12 = np.stack([np.asarray(b1, np.float32), np.asarray(b2, np.float32)], axis=1)
    ident = np.zeros((128, 64), np.float32)
    ident[np.arange(128), np.arange(128) % 64] = 1.0
    # neghalf[:, hc:hc+64] = -0.5*I placed on rows hc:hc+64, zeros elsewhere
    neghalf = np.zeros((128, 128), np.float32)
    neghalf[np.arange(64), np.arange(64)] = -0.5
    neghalf[np.arange(64, 128), np.arange(64, 128)] = -0.5
    ngrp = cfg.PSHARD // 128

    in_maps = []
    for c in range(cfg.NCORES):
        lo, hi = c * cfg.SHARD, min((c + 1) * cfg.SHARD, N)
        ns = hi - lo
        table0[c * cfg.PSHARD:c * cfg.PSHARD + ns] = x[lo:hi].astype(np.float16)
    for c in range(cfg.NCORES):
        lo, hi = c * cfg.SHARD, min((c + 1) * cfg.SHARD, N)
        ns = hi - lo
        x_fm = np.zeros((64, cfg.PSHARD), np.float32)
        x_fm[:, :ns] = x[lo:hi].T
        bt = np.zeros((128, ngrp * NG), np.float16)
        l_ = np.arange(ns)
        bt[l_ % 128, (l_ // 128) * NG + batch[lo:hi]] = 1.0
        in_maps.append({
            "x_fm": x_fm,
            "table0": table0,
            "s_all": plan["S"][c],
            "idx_all": plan["idx"][c],
            "bt_in": bt,
            "w_sb_in": W_sb,
            "b12_in": b12,
            "wfc_in": np.asarray(Wfc, np.float32),
            "bfc_in": np.asarray(bfc, np.float32).reshape(cfg.OUT, 1),
            "ident_in": ident,
            "neghalf_in": neghalf,
        })
    return in_maps


# ---------------------------------------------------------------- device


def build_kernel(cfg, plan, nprop=None, debug=False):
    import concourse.bass as bass
    import concourse.bacc as bacc
    import concourse.mybir as mybir
    import concourse.tile as tile

    dt = mybir.dt
    F, K, NG, OUT = cfg.F, cfg.K, cfg.NG, cfg.OUT
    PSH, TBL, CT, WIN = cfg.PSHARD, cfg.TBL, plan["CT"], cfg.WIN
    NBLK = cfg.NBLK
    ngrp = PSH // 128

    nc = bacc.Bacc("TRN2", debug=False, target_bir_lowering=False,
                   num_devices=cfg.NCORES,
                   dynamic_dma_scratch_size=65536)

    # I/O
    x_fm_t = nc.dram_tensor("x_fm", [64, PSH], dt.float32, kind="ExternalInput")
    table0_t = nc.dram_tensor("table0", [TBL, F], dt.float16, kind="ExternalInput")
    s_all_t = nc.dram_tensor("s_all", [128, CT * 2 * WIN], dt.float16, kind="ExternalInput")
    idx_all_t = nc.dram_tensor("idx_all", [128, CT * 8], dt.int16, kind="ExternalInput")
    bt_t = nc.dram_tensor("bt_in", [128, ngrp * NG], dt.float16, kind="ExternalInput")
    w_sb_t = nc.dram_tensor("w_sb_in", [128, 2 * K * F], dt.float32, kind="ExternalInput")
    b12_t = nc.dram_tensor("b12_in", [64, 2], dt.float32, kind="ExternalInput")
    wfc_t = nc.dram_tensor("wfc_in", [64, OUT], dt.float32, kind="ExternalInput")
    bfc_t = nc.dram_tensor("bfc_in", [OUT, 1], dt.float32, kind="ExternalInput")
    ident_t = nc.dram_tensor("ident_in", [128, 64], dt.float32, kind="ExternalInput")
    neghalf_t = nc.dram_tensor("neghalf_in", [128, 128], dt.float32, kind="ExternalInput")
    out_t = nc.dram_tensor("out_t", [OUT, NG], dt.float32, kind="ExternalOutput")
    if debug:
        dbg0_t = nc.dram_tensor("dbg0", [64, PSH], dt.float32, kind="ExternalOutput")
        dbg1_t = nc.dram_tensor("dbg1", [64, PSH], dt.float32, kind="ExternalOutput")
        dbgo_t = nc.dram_tensor("dbgo", [64, PSH], dt.float32, kind="ExternalOutput")

    f32r = dt.float32r
    rg = [list(range(cfg.NCORES))]
    skip_gather = bool(int(os.environ.get("KSKIP_GATHER", "0")))
    skip_ag = bool(int(os.environ.get("KSKIP_AG", "0")))
    skip_trans = bool(int(os.environ.get("KSKIP_TRANS", "0")))

    with tile.TileContext(nc) as tc:
        with (
            tc.tile_pool(name="const", bufs=1) as cpool,
            tc.tile_pool(name="state", bufs=1) as spool,
            tc.tile_pool(name="gather", bufs=3) as gpool,
            tc.tile_pool(name="smat", bufs=3) as smpool,
            tc.tile_pool(name="idx", bufs=3) as ipool,
            tc.tile_pool(name="psum_y", bufs=2, space="PSUM") as pyp,
            tc.tile_pool(name="psum_w", bufs=2, space="PSUM") as pwp,
            tc.tile_pool(name="psum_t", bufs=2, space="PSUM") as ptp,
            tc.tile_pool(name="dram", bufs=1, space="DRAM") as dpool,
        ):
            # ---- constants to SBUF
            w_sb = cpool.tile([128, 2 * K * F], dt.float32)
            b12_sb = cpool.tile([64, 2], dt.float32)
            wfc_sb = cpool.tile([64, OUT], dt.float32)
            bfc_sb = cpool.tile([OUT, 1], dt.float32)
            ident_sb = cpool.tile([128, 64], dt.float32)
            neghalf_sb = cpool.tile([128, 128], dt.float32)
            bt_sb = cpool.tile([128, ngrp * NG], dt.float16)
            nc.sync.dma_start(out=w_sb[:], in_=w_sb_t[:])
            nc.sync.dma_start(out=b12_sb[:], in_=b12_t[:])
            nc.sync.dma_start(out=wfc_sb[:], in_=wfc_t[:])
            nc.sync.dma_start(out=bfc_sb[:], in_=bfc_t[:])
            nc.sync.dma_start(out=ident_sb[:], in_=ident_t[:])
            nc.sync.dma_start(out=neghalf_sb[:], in_=neghalf_t[:])
            nc.sync.dma_start(out=bt_sb[:], in_=bt_t[:])

            # ---- state
            stA = spool.tile([128, PSH], dt.float32)   # halves: Tx(k even) / Tx(k odd)
            out_sb = spool.tile([64, PSH], dt.float32)
            nm_sb = spool.tile([128, ngrp * F], dt.float16)
            p_sb = spool.tile([64, 512], dt.float32)   # selu pos part scratch
            g_sb = spool.tile([64, NG], dt.float32)
            gfull_sb = spool.tile([64, NG], dt.float32)
            o_sb = spool.tile([OUT, NG], dt.float32)

            nc.sync.dma_start(out=stA[0:64, :], in_=x_fm_t[:])

            # ---- DRAM
            tbuf0 = dpool.tile([TBL, F], dt.float16, tag="tbuf0")
            tbuf1 = dpool.tile([TBL, F], dt.float16, tag="tbuf1")
            tbuf = [tbuf0, tbuf1]
            stage_in = dpool.tile([PSH, F], dt.float16)
            gt_in = dpool.tile([64, NG], dt.float32)
            gt_out = dpool.tile([64, NG], dt.float32)

            def gather_src(h, bk):
                """in_ap for dma_gather: bucket bk of the table for prop h."""
                base = 0 if bk == 0 else 65536
                rows = min(TBL, 65536) if bk == 0 else TBL - 65536
                t = table0_t if h == 0 else tbuf[h % 2]
                ap = t[base:base + rows, :] if h == 0 else t[base:base + rows, :]
                return ap.rearrange("(r two) f -> r (two f)", two=2)

            NPROP = 2 * (K - 1) if nprop is None else nprop
            for h in range(NPROP):
                l, k = h // (K - 1), h % (K - 1) + 1
                hc = (k % 2) * 64          # partition base of Tx_k
                hp = 64 - hc               # partition base of Tx_{k-2 / k-1}

                for b in range(NBLK):
                    w_b = min(512, PSH - b * 512)
                    bc = slice(b * 512, b * 512 + w_b)
                    psum_y = pyp.tile([128, 512], dt.float32)
                    if k == 1:
                        nc.vector.memset(psum_y[hc:hc + 64, :w_b], 0.0)
                    else:
                        # psum := -Tx_{k-2}/2  (Tx_{k-2} shares half hc with Tx_k;
                        # K=128 with zero rows keeps tile_position uniform)
                        nc.tensor.matmul(
                            psum_y[hc:hc + 64, :w_b],
                            neghalf_sb[:, hc:hc + 64],
                            stA[:, bc],
                            start=True, stop=False, skip_group_check=True)
                    for bk in range(cfg.NBUCKET if not skip_gather else 0):
                        bm = plan["blocks"][b][bk]
                        src_ap = gather_src(h, bk)
                        for (cs, nch) in bm["calls"]:
                            it = ipool.tile([128, cfg.CALL_CHUNKS * 8], dt.int16)
                            st = smpool.tile([128, cfg.CALL_CHUNKS * 2 * WIN], dt.float16)
                            gt = gpool.tile([128, cfg.CALL_CHUNKS * 128], dt.float16)
                            nc.sync.dma_start(out=it[:, :nch * 8],
                                              in_=idx_all_t[:, cs * 8:(cs + nch) * 8])
                            nc.sync.dma_start(
                                out=st[:, :nch * 2 * WIN],
                                in_=s_all_t[:, cs * 2 * WIN:(cs + nch) * 2 * WIN])
                            L = nch * 128
                            nc.gpsimd.dma_gather(
                                gt[:, :L].rearrange("p (c f) -> p c f", f=128),
                                src_ap, it[:, :nch * 8], L, L, 128,
                                single_packet=False)
                            for j in range(nch):
                                w = bm["wins"][cs - bm["cstart"] + j]
                                last = (bk == cfg.NBUCKET - 1
                                        and cs + j == bm["cstart"] + bm["nch"] - 1)
                                c0 = j * 128
                                sb = j * 2 * WIN
                                nc.tensor.matmul(
                                    psum_y[hc:hc + 64, w:w + WIN],
                                    gt[:, c0:c0 + 64],
                                    st[:, sb:sb + WIN],
                                    start=False, stop=False, skip_group_check=True)
                                nc.tensor.matmul(
                                    psum_y[hc:hc + 64, w:w + WIN],
                                    gt[:, c0 + 64:c0 + 128],
                                    st[:, sb + WIN:sb + 2 * WIN],
                                    start=False, stop=last, skip_group_check=True)
                    # evacuate: Tx_k
                    if k == 1:
                        nc.vector.tensor_copy(stA[hc:hc + 64, bc], psum_y[hc:hc + 64, :w_b])
                    else:
                        nc.vector.tensor_scalar(stA[hc:hc + 64, bc], psum_y[hc:hc + 64, :w_b],
                                                2.0, None, mybir.AluOpType.mult)
                    # out += Tx_k @ W_k  (transposed: psum_w = W_k.T @ Tx_k)
                    psum_w = pwp.tile([64, 512], dt.float32)
                    wc = (l * K + k) * F
                    nc.tensor.matmul(psum_w[:, :w_b],
                                     w_sb[:, wc:wc + F],
                                     stA[:, bc],
                                     start=True, stop=(k != 1), skip_group_check=True)
                    if k == 1:  # also Tx0 @ W0
                        nc.tensor.matmul(psum_w[:, :w_b],
                                         w_sb[:, (l * K) * F:(l * K) * F + F],
                                         stA[:, bc],
                                         start=False, stop=True, skip_group_check=True)
                        nc.vector.tensor_copy(out_sb[:, bc], psum_w[:, :w_b])
                    else:
                        nc.vector.tensor_tensor(out_sb[:, bc], out_sb[:, bc],
                                                psum_w[:, :w_b], mybir.AluOpType.add)

                hsrc = hc  # partition base of features to tableize
                if k == K - 1:
                    # ---- h = selu(out + b_l) -> stA[0:64]
                    lam, alpha = SELU_L, SELU_A
                    for b in range(NBLK):
                        w_b = min(512, PSH - b * 512)
                        bc = slice(b * 512, b * 512 + w_b)
                        nc.vector.tensor_scalar(out_sb[:, bc], out_sb[:, bc],
                                                b12_sb[:, l:l + 1], None,
                                                mybir.AluOpType.add)
                        nc.scalar.activation(p_sb[:, :w_b], out_sb[:, bc],
                                             mybir.ActivationFunctionType.Relu,
                                             scale=lam)
                        nc.vector.tensor_scalar(out_sb[:, bc], out_sb[:, bc],
                                                0.0, None, mybir.AluOpType.min)
                        nc.scalar.activation(out_sb[:, bc], out_sb[:, bc],
                                             mybir.ActivationFunctionType.Exp)
                        nc.vector.tensor_scalar(out_sb[:, bc], out_sb[:, bc],
                                                lam * alpha, -lam * alpha,
                                                mybir.AluOpType.mult,
                                                mybir.AluOpType.add)
                        nc.vector.tensor_tensor(stA[0:64, bc], out_sb[:, bc],
                                                p_sb[:, :w_b], mybir.AluOpType.add)
                    hsrc = 0

                # ---- build node-major fp16 table from stA[hsrc:hsrc+64]
                if not skip_trans:
                    for g in range(ngrp):
                        pt = ptp.tile([128, 64], dt.float32)
                        nc.tensor.matmul(pt[:],
                                         stA[hsrc:hsrc + 64, g * 128:(g + 1) * 128],
                                         ident_sb[hsrc:hsrc + 64, :],
                                         is_transpose=True, skip_group_check=True)
                        nc.vector.tensor_copy(nm_sb[:, g * F:(g + 1) * F], pt[:])
                if not (l == 1 and k == K - 1) and not skip_ag and not skip_trans:
                    # ship shard and all-gather into the other table buffer
                    nc.sync.dma_start(
                        out=stage_in[:].rearrange("(g p) f -> p g f", p=128),
                        in_=nm_sb[:].rearrange("p (g f) -> p g f", f=F))
                    nc.gpsimd.collective_compute(
                        "AllGather", mybir.AluOpType.bypass,
                        replica_groups=rg,
                        ins=[stage_in.opt()],
                        outs=[tbuf[(h + 1) % 2].opt()])

            if debug:
                nc.sync.dma_start(out=dbg0_t[:], in_=stA[0:64, :])
                nc.sync.dma_start(out=dbg1_t[:], in_=stA[64:128, :])
                nc.sync.dma_start(out=dbgo_t[:], in_=out_sb[:])
            # ---- pooling: gT = sum_n h2[n] per graph  (psum[64f, NG])
            if not skip_trans:
                psum_g = pwp.tile([64, 512], dt.float32, tag="psum_w")
                for g in range(ngrp):
                    nc.tensor.matmul(psum_g[:, :NG],
                                     nm_sb[:, g * F:(g + 1) * F],
                                     bt_sb[:, g * NG:(g + 1) * NG],
                                     start=(g == 0), stop=(g == ngrp - 1),
                                     skip_group_check=True)
                nc.vector.tensor_copy(g_sb[:], psum_g[:, :NG])
            else:
                nc.vector.memset(g_sb[:], 0.0)
            nc.sync.dma_start(out=gt_in[:], in_=g_sb[:])
            nc.gpsimd.collective_compute(
                "AllReduce", mybir.AluOpType.add, replica_groups=rg,
                ins=[gt_in.opt()], outs=[gt_out.opt()])
            nc.sync.dma_start(out=gfull_sb[:], in_=gt_out[:])
            psum_o = ptp.tile([128, 64], dt.float32, tag="pt")
            nc.tensor.matmul(psum_o[0:OUT, 0:NG],
                             wfc_sb[:],
                             gfull_sb[:],
                             start=True, stop=True, skip_group_check=True)
            nc.vector.tensor_scalar(o_sb[:], psum_o[0:OUT, 0:NG],
                                    bfc_sb[:, 0:1], None, mybir.AluOpType.add)
            nc.sync.dma_start(out=out_t[:], in_=o_sb[:])

    nc.compile()
    return nc


# ---------------------------------------------------------------- entry


def run(cfg, inputs, trace=False):
    from concourse.bass_utils import run_bass_kernel_spmd
    edge_index = np.asarray(inputs["edge_index"])
    plan = build_plan(cfg, edge_index)
    nprop = int(os.environ.get("KNPROP", "0")) or None
    nc = build_kernel(cfg, plan, nprop=nprop)
    in_maps = build_host_inputs(
        cfg, plan, inputs["x"], inputs["batch"],
        inputs["W1"], inputs["b1"], inputs["W2"], inputs["b2"],
        inputs["Wfc"], inputs["bfc"])
    core_ids = list(range(cfg.NCORES))
    res = run_bass_kernel_spmd(nc, in_maps, core_ids, trace=trace)
    out = np.asarray(res.results[0]["out_t"]).T.copy()  # [NG, OUT]
    return out, res


def kernel(**inputs):
    cfg = Cfg()
    out, _ = run(cfg, inputs, trace=False)
    return out.astype(np.float32)



# revision 13
# speedup vs baseline: 1.1249x; 1.0166x over previous
"""Trainium2 Bass kernel for a 2-layer ChebConv (K=5) GNN + global_add_pool + fc.

Strategy (8 NeuronCores, SPMD):
  - dst-shard the edges: core c owns all edges whose dst lands in its node
    shard. Each hop's scatter output is then complete per-core (no
    all-reduce); cores exchange an fp16 node-feature table via AllGather.
  - Gather messages with dma_gather (256B elements = two consecutive fp16
    rows of the node table; per-edge parity selects which half, handled by
    splitting each 128-edge chunk into two 64-row K-ranges on the PE).
  - Scatter-add via PE matmul: per chunk, out[feat, node_window] +=
    G.T @ S where S is a host-precomputed fp16 selection matrix with the
    edge weight (-1/deg[src]) folded in.
  - Chebyshev recurrence (Tx2 = 2*prop(Tx1) - Tx0) is fused: the PSUM is
    seeded with -Tx0/2 via an f32r matmul and the evacuation multiplies
    by 2.  Per-node W matmuls run as f32r with the fp32 state as rhs.
"""

import os
import sys
import numpy as np

for _p in ("/opt/trn_rl_repo",):
    if os.path.isdir(_p) and _p not in sys.path:
        sys.path.insert(0, _p)

# ---------------------------------------------------------------- config

SELU_L = 1.0507009873554805
SELU_A = 1.6732632423543772


class Cfg:
    def __init__(self, N=100_000, E=1_250_000, NG=64, F=64, K=5, OUT=10,
                 NCORES=8, call_chunks=32):
        self.N, self.E, self.NG, self.F, self.K, self.OUT = N, E, NG, F, K, OUT
        self.NCORES = NCORES
        self.SHARD = (N + NCORES - 1) // NCORES
        self.PSHARD = ((self.SHARD + 127) // 128) * 128
        self.TBL = NCORES * self.PSHARD          # padded table rows
        self.NBLK = (self.PSHARD + 511) // 512   # psum blocks per shard
        self.WIN = 64                            # scatter window (nodes)
        self.NBUCKET = 2 if self.TBL > 65536 else 1
        self.CALL_CHUNKS = call_chunks           # chunks per dma_gather call


# ---------------------------------------------------------------- host plan


def build_plan(cfg, edge_index):
    """Global (core-independent) chunk structure + per-core S/idx arrays."""
    N, NC = cfg.N, cfg.NCORES
    src = np.asarray(edge_index[0], dtype=np.int64)
    dst = np.asarray(edge_index[1], dtype=np.int64)
    deg = np.bincount(src, minlength=N).astype(np.float64)
    ew = (-1.0 / deg[src]).astype(np.float32)

    # table row of each src node (shards padded to PSHARD rows)
    r_src = (src // cfg.SHARD) * cfg.PSHARD + (src % cfg.SHARD)
    core = dst // cfg.SHARD
    dl = dst % cfg.SHARD                      # dst local node id
    blk = dl // 512
    par = (r_src & 1).astype(np.int64)        # parity within row pair
    pair = r_src >> 1
    bkt = (pair >= 32768).astype(np.int64) if cfg.NBUCKET == 2 else np.zeros_like(par)
    m = np.where(bkt == 1, pair - 32768, pair).astype(np.int64)  # bucket-local

    # group edges by (core, block, bucket), dst-sorted inside; chunks mix
    # parities (per-slot parity masks select the gathered pair half on-chip)
    key = ((core * cfg.NBLK + blk) * cfg.NBUCKET + bkt) * cfg.SHARD + dl
    order = np.argsort(key, kind="stable")
    g_dl, g_m, g_ew, g_par = dl[order], m[order], ew[order], par[order]
    gk = key[order] // cfg.SHARD  # group id per sorted edge
    ngroups = NC * cfg.NBLK * cfg.NBUCKET
    starts = np.searchsorted(gk, np.arange(ngroups + 1))

    def grp(c, b, bk):
        gid = (c * cfg.NBLK + b) * cfg.NBUCKET + bk
        s, e = starts[gid], starts[gid + 1]
        return g_dl[s:e], g_m[s:e], g_ew[s:e], g_par[s:e]

    blocks_meta = []   # [b][bkt] -> dict(calls=[(cstart,nch)], wins=[w...])
    CT = 0
    # per-core chunk piece lists: (chunk_id, dsts, ms, ews, pars) appended
    core_chunks = [[] for _ in range(NC)]

    for b in range(cfg.NBLK):
        bmeta = []
        for bk in range(cfg.NBUCKET):
            data = [grp(c, b, bk) for c in range(NC)]
            ptr = [0 for _ in range(NC)]
            wins = []   # w0 per chunk
            run_start = CT
            while True:
                wmin = None
                for c in range(NC):
                    d = data[c][0]
                    if ptr[c] < len(d):
                        v = d[ptr[c]]
                        if wmin is None or v < wmin:
                            wmin = v
                if wmin is None:
                    break
                wb_psum = min(512, cfg.PSHARD - b * 512)
                w0 = min(int(wmin) - b * 512,      # window base within block
                         max(0, wb_psum - cfg.WIN))
                limit = b * 512 + w0 + cfg.WIN
                cid = CT
                for c in range(NC):
                    d, mm, ee, pp = data[c]
                    lo = ptr[c]
                    hi = np.searchsorted(d, limit, side="left")
                    take = min(128, hi - lo)
                    if take > 0:
                        core_chunks[c].append(
                            (cid, d[lo:lo + take] - b * 512 - w0,
                             mm[lo:lo + take], ee[lo:lo + take],
                             pp[lo:lo + take]))
                        ptr[c] = lo + take
                wins.append(w0)
                CT += 1
            nch_run = CT - run_start
            calls = []
            off = 0
            while off < nch_run:
                n = min(cfg.CALL_CHUNKS, nch_run - off)
                calls.append((run_start + off, n))
                off += n
            bmeta.append({"calls": calls, "wins": wins, "cstart": run_start,
                          "nch": nch_run})
        blocks_meta.append(bmeta)

    # materialize per-core arrays
    S_list, idx_list = [], []
    for c in range(NC):
        S = np.zeros((CT, 2, 128, cfg.WIN), dtype=np.float16)
        idx = np.zeros((CT, 128), dtype=np.int16)
        for cid, dls, ms, ees, pps in core_chunks[c]:
            n = len(dls)
            rows = np.arange(n)
            S[cid, pps, rows, dls] = ees.astype(np.float16)
            idx[cid, rows] = ms.astype(np.int16)
        # wrap idx per call: position i -> [i%16, i//16]
        idx_w = np.zeros((16, CT * 8), dtype=np.int16)
        for bmeta in blocks_meta:
            for bm in bmeta:
                for (cs, n) in bm["calls"]:
                    flat = idx[cs:cs + n].reshape(-1)           # [n*128]
                    idx_w[:, cs * 8:(cs + n) * 8] = flat.reshape(-1, 16).T
        # S flat: [128, CT*2*WIN] (even-parity S then odd-parity S per chunk)
        S_flat = np.ascontiguousarray(
            S.transpose(2, 0, 1, 3).reshape(128, CT * 2 * cfg.WIN))
        S_list.append(S_flat)
        idx_list.append(np.ascontiguousarray(np.tile(idx_w, (8, 1))))  # [128, CT*8]

    return {"CT": CT, "blocks": blocks_meta, "S": S_list, "idx": idx_list}


def build_host_inputs(cfg, plan, x, batch, W1, b1, W2, b2, Wfc, bfc):
    """Per-core in_map dicts."""
    N, F, NG = cfg.N, cfg.F, cfg.NG
    x = np.asarray(x, np.float32)
    batch = np.asarray(batch, np.int64)
    table0 = np.zeros((cfg.TBL, F), np.float16)
    W_sb = np.zeros((128, 2 * cfg.K * F), np.float32)
    for l, W in enumerate((W1, W2)):
        for k in range(cfg.K):
            blkc = (l * cfg.K + k) * F
            r0 = (k % 2) * 64          # W_k contracts state rows of Tx_k
            W_sb[r0:r0 + 64, blkc:blkc + F] = W[k]
    b12 = np.stack([np.asarray(b1, np.float32), np.asarray(b2, np.float32)], axis=1)
    ident = np.zeros((128, 64), np.float32)
    ident[np.arange(128), np.arange(128) % 64] = 1.0
    # neghalf[:, hc:hc+64] = -0.5*I placed on rows hc:hc+64, zeros elsewhere
    neghalf = np.zeros((128, 128), np.float32)
    neghalf[np.arange(64), np.arange(64)] = -0.5
    neghalf[np.arange(64, 128), np.arange(64, 128)] = -0.5
    ngrp = cfg.PSHARD // 128

    in_maps = []
    for c in range(cfg.NCORES):
        lo, hi = c * cfg.SHARD, min((c + 1) * cfg.SHARD, N)
        ns = hi - lo
        table0[c * cfg.PSHARD:c * cfg.PSHARD + ns] = x[lo:hi].astype(np.float16)
    for c in range(cfg.NCORES):
        lo, hi = c * cfg.SHARD, min((c + 1) * cfg.SHARD, N)
        ns = hi - lo
        x_fm = np.zeros((64, cfg.PSHARD), np.float32)
        x_fm[:, :ns] = x[lo:hi].T
        bt = np.zeros((128, ngrp * NG), np.float16)
        l_ = np.arange(ns)
        bt[l_ % 128, (l_ // 128) * NG + batch[lo:hi]] = 1.0
        in_maps.append({
            "x_fm": x_fm,
            "table0": table0,
            "s_all": plan["S"][c],
            "idx_all": plan["idx"][c],
            "bt_in": bt,
            "w_sb_in": W_sb,
            "b12_in": b12,
            "wfc_in": np.asarray(Wfc, np.float32),
            "bfc_in": np.asarray(bfc, np.float32).reshape(cfg.OUT, 1),
            "ident_in": ident,
            "neghalf_in": neghalf,
        })
    return in_maps


# ---------------------------------------------------------------- device


def build_kernel(cfg, plan, nprop=None, debug=False):
    import concourse.bass as bass
    import concourse.bacc as bacc
    import concourse.mybir as mybir
    import concourse.tile as tile

    dt = mybir.dt
    F, K, NG, OUT = cfg.F, cfg.K, cfg.NG, cfg.OUT
    PSH, TBL, CT, WIN = cfg.PSHARD, cfg.TBL, plan["CT"], cfg.WIN
    NBLK = cfg.NBLK
    ngrp = PSH // 128

    nc = bacc.Bacc("TRN2", debug=False, target_bir_lowering=False,
                   num_devices=cfg.NCORES,
                   dynamic_dma_scratch_size=40960)

    # I/O
    x_fm_t = nc.dram_tensor("x_fm", [64, PSH], dt.float32, kind="ExternalInput")
    table0_t = nc.dram_tensor("table0", [TBL, F], dt.float16, kind="ExternalInput")
    s_all_t = nc.dram_tensor("s_all", [128, CT * 2 * WIN], dt.float16, kind="ExternalInput")
    idx_all_t = nc.dram_tensor("idx_all", [128, CT * 8], dt.int16, kind="ExternalInput")
    bt_t = nc.dram_tensor("bt_in", [128, ngrp * NG], dt.float16, kind="ExternalInput")
    w_sb_t = nc.dram_tensor("w_sb_in", [128, 2 * K * F], dt.float32, kind="ExternalInput")
    b12_t = nc.dram_tensor("b12_in", [64, 2], dt.float32, kind="ExternalInput")
    wfc_t = nc.dram_tensor("wfc_in", [64, OUT], dt.float32, kind="ExternalInput")
    bfc_t = nc.dram_tensor("bfc_in", [OUT, 1], dt.float32, kind="ExternalInput")
    ident_t = nc.dram_tensor("ident_in", [128, 64], dt.float32, kind="ExternalInput")
    neghalf_t = nc.dram_tensor("neghalf_in", [128, 128], dt.float32, kind="ExternalInput")
    out_t = nc.dram_tensor("out_t", [OUT, NG], dt.float32, kind="ExternalOutput")
    if debug:
        dbg0_t = nc.dram_tensor("dbg0", [64, PSH], dt.float32, kind="ExternalOutput")
        dbg1_t = nc.dram_tensor("dbg1", [64, PSH], dt.float32, kind="ExternalOutput")
        dbgo_t = nc.dram_tensor("dbgo", [64, PSH], dt.float32, kind="ExternalOutput")

    f32r = dt.float32r
    rg = [list(range(cfg.NCORES))]
    skip_gather = bool(int(os.environ.get("KSKIP_GATHER", "0")))
    skip_ag = bool(int(os.environ.get("KSKIP_AG", "0")))
    skip_trans = bool(int(os.environ.get("KSKIP_TRANS", "0")))

    with tile.TileContext(nc) as tc:
        with (
            tc.tile_pool(name="const", bufs=1) as cpool,
            tc.tile_pool(name="state", bufs=1) as spool,
            tc.tile_pool(name="gather", bufs=3) as gpool,
            tc.tile_pool(name="smat", bufs=3) as smpool,
            tc.tile_pool(name="idx", bufs=3) as ipool,
            tc.tile_pool(name="psum_y", bufs=2, space="PSUM") as pyp,
            tc.tile_pool(name="psum_w", bufs=2, space="PSUM") as pwp,
            tc.tile_pool(name="psum_t", bufs=2, space="PSUM") as ptp,
            tc.tile_pool(name="dram", bufs=1, space="DRAM") as dpool,
        ):
            # ---- constants to SBUF
            w_sb = cpool.tile([128, 2 * K * F], dt.float32)
            b12_sb = cpool.tile([64, 2], dt.float32)
            wfc_sb = cpool.tile([64, OUT], dt.float32)
            bfc_sb = cpool.tile([OUT, 1], dt.float32)
            ident_sb = cpool.tile([128, 64], dt.float32)
            neghalf_sb = cpool.tile([128, 128], dt.float32)
            bt_sb = cpool.tile([128, ngrp * NG], dt.float16)
            nc.sync.dma_start(out=w_sb[:], in_=w_sb_t[:])
            nc.sync.dma_start(out=b12_sb[:], in_=b12_t[:])
            nc.sync.dma_start(out=wfc_sb[:], in_=wfc_t[:])
            nc.sync.dma_start(out=bfc_sb[:], in_=bfc_t[:])
            nc.sync.dma_start(out=ident_sb[:], in_=ident_t[:])
            nc.sync.dma_start(out=neghalf_sb[:], in_=neghalf_t[:])
            nc.sync.dma_start(out=bt_sb[:], in_=bt_t[:])

            # ---- state
            stA = spool.tile([128, PSH], dt.float32)   # halves: Tx(k even) / Tx(k odd)
            out_sb = spool.tile([64, PSH], dt.float32)
            nm_sb = spool.tile([128, ngrp * F], dt.float16)
            p_sb = spool.tile([64, 512], dt.float32)   # selu pos part scratch
            g_sb = spool.tile([64, NG], dt.float32)
            gfull_sb = spool.tile([64, NG], dt.float32)
            o_sb = spool.tile([OUT, NG], dt.float32)

            nc.sync.dma_start(out=stA[0:64, :], in_=x_fm_t[:])

            # ---- DRAM
            tbuf0 = dpool.tile([TBL, F], dt.float16, tag="tbuf0")
            tbuf1 = dpool.tile([TBL, F], dt.float16, tag="tbuf1")
            tbuf = [tbuf0, tbuf1]
            stage_in = dpool.tile([PSH, F], dt.float16)
            gt_in = dpool.tile([64, NG], dt.float32)
            gt_out = dpool.tile([64, NG], dt.float32)

            def gather_src(h, bk):
                """in_ap for dma_gather: bucket bk of the table for prop h."""
                base = 0 if bk == 0 else 65536
                rows = min(TBL, 65536) if bk == 0 else TBL - 65536
                t = table0_t if h == 0 else tbuf[h % 2]
                ap = t[base:base + rows, :] if h == 0 else t[base:base + rows, :]
                return ap.rearrange("(r two) f -> r (two f)", two=2)

            NPROP = 2 * (K - 1) if nprop is None else nprop
            for h in range(NPROP):
                l, k = h // (K - 1), h % (K - 1) + 1
                hc = (k % 2) * 64          # partition base of Tx_k
                hp = 64 - hc               # partition base of Tx_{k-2 / k-1}

                for b in range(NBLK):
                    w_b = min(512, PSH - b * 512)
                    bc = slice(b * 512, b * 512 + w_b)
                    psum_y = pyp.tile([128, 512], dt.float32)
                    if k == 1:
                        nc.vector.memset(psum_y[hc:hc + 64, :w_b], 0.0)
                    else:
                        # psum := -Tx_{k-2}/2  (Tx_{k-2} shares half hc with Tx_k;
                        # K=128 with zero rows keeps tile_position uniform)
                        nc.tensor.matmul(
                            psum_y[hc:hc + 64, :w_b],
                            neghalf_sb[:, hc:hc + 64],
                            stA[:, bc],
                            start=True, stop=False, skip_group_check=True)
                    for bk in range(cfg.NBUCKET if not skip_gather else 0):
                        bm = plan["blocks"][b][bk]
                        src_ap = gather_src(h, bk)
                        for (cs, nch) in bm["calls"]:
                            it = ipool.tile([128, cfg.CALL_CHUNKS * 8], dt.int16)
                            st = smpool.tile([128, cfg.CALL_CHUNKS * 2 * WIN], dt.float16)
                            gt = gpool.tile([128, cfg.CALL_CHUNKS * 128], dt.float16)
                            nc.sync.dma_start(out=it[:, :nch * 8],
                                              in_=idx_all_t[:, cs * 8:(cs + nch) * 8])
                            nc.scalar.dma_start(
                                out=st[:, :nch * 2 * WIN],
                                in_=s_all_t[:, cs * 2 * WIN:(cs + nch) * 2 * WIN])
                            L = nch * 128
                            nc.gpsimd.dma_gather(
                                gt[:, :L].rearrange("p (c f) -> p c f", f=128),
                                src_ap, it[:, :nch * 8], L, L, 128,
                                single_packet=False)
                            for j in range(nch):
                                w = bm["wins"][cs - bm["cstart"] + j]
                                last = (bk == cfg.NBUCKET - 1
                                        and cs + j == bm["cstart"] + bm["nch"] - 1)
                                c0 = j * 128
                                sb = j * 2 * WIN
                                nc.tensor.matmul(
                                    psum_y[hc:hc + 64, w:w + WIN],
                                    gt[:, c0:c0 + 64],
                                    st[:, sb:sb + WIN],
                                    start=False, stop=False, skip_group_check=True)
                                nc.tensor.matmul(
                                    psum_y[hc:hc + 64, w:w + WIN],
                                    gt[:, c0 + 64:c0 + 128],
                                    st[:, sb + WIN:sb + 2 * WIN],
                                    start=False, stop=last, skip_group_check=True)
                    # evacuate: Tx_k
                    if k == 1:
                        nc.vector.tensor_copy(stA[hc:hc + 64, bc], psum_y[hc:hc + 64, :w_b])
                    else:
                        nc.vector.tensor_scalar(stA[hc:hc + 64, bc], psum_y[hc:hc + 64, :w_b],
                                                2.0, None, mybir.AluOpType.mult)
                    # out += Tx_k @ W_k  (transposed: psum_w = W_k.T @ Tx_k)
                    psum_w = pwp.tile([64, 512], dt.float32)
                    wc = (l * K + k) * F
                    nc.tensor.matmul(psum_w[:, :w_b],
                                     w_sb[:, wc:wc + F],
                                     stA[:, bc],
                                     start=True, stop=(k != 1), skip_group_check=True)
                    if k == 1:  # also Tx0 @ W0
                        nc.tensor.matmul(psum_w[:, :w_b],
                                         w_sb[:, (l * K) * F:(l * K) * F + F],
                                         stA[:, bc],
                                         start=False, stop=True, skip_group_check=True)
                        nc.vector.tensor_copy(out_sb[:, bc], psum_w[:, :w_b])
                    else:
                        nc.vector.tensor_tensor(out_sb[:, bc], out_sb[:, bc],
                                                psum_w[:, :w_b], mybir.AluOpType.add)

                hsrc = hc  # partition base of features to tableize
                if k == K - 1:
                    # ---- h = selu(out + b_l) -> stA[0:64]
                    lam, alpha = SELU_L, SELU_A
                    for b in range(NBLK):
                        w_b = min(512, PSH - b * 512)
                        bc = slice(b * 512, b * 512 + w_b)
                        nc.vector.tensor_scalar(out_sb[:, bc], out_sb[:, bc],
                                                b12_sb[:, l:l + 1], None,
                                                mybir.AluOpType.add)
                        nc.scalar.activation(p_sb[:, :w_b], out_sb[:, bc],
                                             mybir.ActivationFunctionType.Relu,
                                             scale=lam)
                        nc.vector.tensor_scalar(out_sb[:, bc], out_sb[:, bc],
                                                0.0, None, mybir.AluOpType.min)
                        nc.scalar.activation(out_sb[:, bc], out_sb[:, bc],
                                             mybir.ActivationFunctionType.Exp)
                        nc.vector.tensor_scalar(out_sb[:, bc], out_sb[:, bc],
                                                lam * alpha, -lam * alpha,
                                                mybir.AluOpType.mult,
                                                mybir.AluOpType.add)
                        nc.vector.tensor_tensor(stA[0:64, bc], out_sb[:, bc],
                                                p_sb[:, :w_b], mybir.AluOpType.add)
                    hsrc = 0

                # ---- build node-major fp16 table from stA[hsrc:hsrc+64]
                if not skip_trans:
                    for g in range(ngrp):
                        pt = ptp.tile([128, 64], dt.float32)
                        nc.tensor.matmul(pt[:],
                                         stA[hsrc:hsrc + 64, g * 128:(g + 1) * 128],
                                         ident_sb[hsrc:hsrc + 64, :],
                                         is_transpose=True, skip_group_check=True)
                        nc.vector.tensor_copy(nm_sb[:, g * F:(g + 1) * F], pt[:])
                if not (l == 1 and k == K - 1) and not skip_ag and not skip_trans:
                    # ship shard and all-gather into the other table buffer
                    nc.sync.dma_start(
                        out=stage_in[:].rearrange("(g p) f -> p g f", p=128),
                        in_=nm_sb[:].rearrange("p (g f) -> p g f", f=F))
                    nc.gpsimd.collective_compute(
                        "AllGather", mybir.AluOpType.bypass,
                        replica_groups=rg,
                        ins=[stage_in.opt()],
                        outs=[tbuf[(h + 1) % 2].opt()])

            if debug:
                nc.sync.dma_start(out=dbg0_t[:], in_=stA[0:64, :])
                nc.sync.dma_start(out=dbg1_t[:], in_=stA[64:128, :])
                nc.sync.dma_start(out=dbgo_t[:], in_=out_sb[:])
            # ---- pooling: gT = sum_n h2[n] per graph  (psum[64f, NG])
            if not skip_trans:
                psum_g = pwp.tile([64, 512], dt.float32, tag="psum_w")
                for g in range(ngrp):
                    nc.tensor.matmul(psum_g[:, :NG],
                                     nm_sb[:, g * F:(g + 1) * F],
                                     bt_sb[:, g * NG:(g + 1) * NG],
                                     start=(g == 0), stop=(g == ngrp - 1),
                                     skip_group_check=True)
                nc.vector.tensor_copy(g_sb[:], psum_g[:, :NG])
            else:
                nc.vector.memset(g_sb[:], 0.0)
            nc.sync.dma_start(out=gt_in[:], in_=g_sb[:])
            nc.gpsimd.collective_compute(
                "AllReduce", mybir.AluOpType.add, replica_groups=rg,
                ins=[gt_in.opt()], outs=[gt_out.opt()])
            nc.sync.dma_start(out=gfull_sb[:], in_=gt_out[:])
            psum_o = ptp.tile([128, 64], dt.float32, tag="pt")
            nc.tensor.matmul(psum_o[0:OUT, 0:NG],
                             wfc_sb[:],
                             gfull_sb[:],
                             start=True, stop=True, skip_group_check=True)
            nc.vector.tensor_scalar(o_sb[:], psum_o[0:OUT, 0:NG],
                                    bfc_sb[:, 0:1], None, mybir.AluOpType.add)
            nc.sync.dma_start(out=out_t[:], in_=o_sb[:])

    nc.compile()
    return nc


# ---------------------------------------------------------------- entry


def run(cfg, inputs, trace=False):
    from concourse.bass_utils import run_bass_kernel_spmd
    edge_index = np.asarray(inputs["edge_index"])
    plan = build_plan(cfg, edge_index)
    nprop = int(os.environ.get("KNPROP", "0")) or None
    nc = build_kernel(cfg, plan, nprop=nprop)
    in_maps = build_host_inputs(
        cfg, plan, inputs["x"], inputs["batch"],
        inputs["W1"], inputs["b1"], inputs["W2"], inputs["b2"],
        inputs["Wfc"], inputs["bfc"])
    core_ids = list(range(cfg.NCORES))
    res = run_bass_kernel_spmd(nc, in_maps, core_ids, trace=trace)
    out = np.asarray(res.results[0]["out_t"]).T.copy()  # [NG, OUT]
    return out, res


def kernel(**inputs):
    cfg = Cfg()
    out, _ = run(cfg, inputs, trace=False)
    return out.astype(np.float32)

